# revision 1
# baseline (speedup 1.0000x reference)
"""Multi-head attention (B=2, S=2048, H=1024, 16 heads) on 8 trn2 NeuronCores.

Sharding: tensor-parallel over heads — each core owns 2 heads (128 channels of
the QKV projections and 128 input channels of the output projection). Every
core consumes the full (transposed, bf16-cast) activations; partial outputs of
the wo projection are summed on the host.

Device-side dataflow per core (all matmuls bf16 with f32 PSUM accumulation):
  QT[c,s] = (wq_c x^T + bq) : transposed projections, channels on partitions
  KT[c,s] likewise; V[s,c] in natural layout (tokens on partitions)
  scores^T[k,q] = KT_h^T-tile . QT_h  (two heads row-packed on the PE array)
  E = exp(scores/8)  (no max subtraction: scores are ~N(0,1), |s| < ~6)
  O^T[d,q], sums[q] accumulate over key tiles via ones-augmented V (M=65)
  O_norm = O^T * bcast(1/sums); y^T partial = woT_c . O_norm
"""

import os
import threading

import numpy as np
import ml_dtypes

import concourse.bass as bass
import concourse.mybir as mybir
import concourse.tile as tile
from concourse import bacc
from concourse.bass_utils import run_bass_kernel_spmd

BF16 = ml_dtypes.bfloat16
F32 = mybir.dt.float32
BF = mybir.dt.bfloat16

B = 2
S = 2048
H = 1024
NS = B * S          # 4096 tokens
NH_LOCAL = 2        # heads per core
HD = 64             # head dim
CPC = 128           # channels per core
NF = H // 128       # feature chunks
N_CORES = 8

_cache = threading.Lock()
_nc = None

LAST_RESULT = None  # BassKernelResults of the most recent run (for test.py)


def _build_nc():
    nc = bacc.Bacc(None, target_bir_lowering=False, debug=False)

    xq_d = nc.dram_tensor("xq_t", [H, NS], BF, kind="ExternalInput")
    xk_d = nc.dram_tensor("xk_t", [H, NS], BF, kind="ExternalInput")
    xv_d = nc.dram_tensor("xv_t", [H, NS], BF, kind="ExternalInput")
    wq_d = nc.dram_tensor("wq_t", [H, CPC], BF, kind="ExternalInput")
    wk_d = nc.dram_tensor("wk_t", [H, CPC], BF, kind="ExternalInput")
    wv_d = nc.dram_tensor("wv_t", [H, CPC], BF, kind="ExternalInput")
    bq_d = nc.dram_tensor("bq", [CPC, 1], F32, kind="ExternalInput")
    bk_d = nc.dram_tensor("bk", [CPC, 1], F32, kind="ExternalInput")
    bv_d = nc.dram_tensor("bv", [1, CPC], BF, kind="ExternalInput")
    wo_d = nc.dram_tensor("wo_t", [CPC, H], BF, kind="ExternalInput")
    y_d = nc.dram_tensor("y_t", [H, NS], F32, kind="ExternalOutput")

    xq_ap = xq_d.rearrange("(nf p) s -> nf p s", p=128)
    xk_ap = xk_d.rearrange("(nf p) s -> nf p s", p=128)
    xv_ap = xv_d.rearrange("(nf p) s -> nf p s", p=128)
    y_ap = y_d.rearrange("(no p) s -> no p s", p=128)

    Exp = mybir.ActivationFunctionType.Exp
    Copy = mybir.ActivationFunctionType.Identity

    with tile.TileContext(nc) as tc:
        with (
            tc.tile_pool(name="const", bufs=1) as const,
            tc.tile_pool(name="res", bufs=1) as res,
            tc.tile_pool(name="work", bufs=3) as work,
            tc.tile_pool(name="psum", bufs=2, space="PSUM") as psum,
        ):
            # --- constants / weights ---
            wq_sb = const.tile([128, NF, CPC], BF)
            wk_sb = const.tile([128, NF, CPC], BF)
            wv_sb = const.tile([128, NF, CPC], BF)
            wo_sb = const.tile([128, NF, 128], BF)
            bq_sb = const.tile([128, 1], F32)
            bk_sb = const.tile([128, 1], F32)
            bv_sb = const.tile([1, CPC], BF)
            ones1 = const.tile([1, 128], BF)
            nc.sync.dma_start(wq_sb[:], wq_d.rearrange("(nf p) c -> p nf c", p=128))
            nc.sync.dma_start(wk_sb[:], wk_d.rearrange("(nf p) c -> p nf c", p=128))
            nc.sync.dma_start(wv_sb[:], wv_d.rearrange("(nf p) c -> p nf c", p=128))
            nc.sync.dma_start(wo_sb[:], wo_d.rearrange("p (no c) -> p no c", c=128))
            nc.sync.dma_start(bq_sb[:], bq_d[:])
            nc.sync.dma_start(bk_sb[:], bk_d[:])
            nc.sync.dma_start(bv_sb[:], bv_d[:])
            nc.gpsimd.memset(ones1[:], 1.0)

            # --- residents ---
            QT = res.tile([128, NS], BF)
            KT = res.tile([128, NS], BF)
            V0 = res.tile([128, NS // 128, HD + 1], BF)
            V1 = res.tile([128, NS // 128, HD + 1], BF)
            nc.gpsimd.memset(V0[:, :, HD : HD + 1], 1.0)
            nc.gpsimd.memset(V1[:, :, HD : HD + 1], 1.0)

            # --- projections ---
            with tc.tile_pool(name="xin", bufs=10) as xin:
                for name, x_ap, w_sb, b_sb, out_t in (
                    ("q", xq_ap, wq_sb, bq_sb, QT),
                    ("k", xk_ap, wk_sb, bk_sb, KT),
                ):
                    xt = []
                    for f in range(NF):
                        t = xin.tile([128, NS], BF, tag="xc", name=f"x{name}{f}")
                        nc.sync.dma_start(t[:], x_ap[f])
                        xt.append(t)
                    for sw in range(NS // 512):
                        ps = psum.tile([128, 512], F32, tag="s", name=f"ps{name}{sw}")
                        for f in range(NF):
                            nc.tensor.matmul(
                                ps[:],
                                lhsT=w_sb[:, f, :],
                                rhs=xt[f][:, sw * 512 : (sw + 1) * 512],
                                start=(f == 0),
                                stop=(f == NF - 1),
                            )
                        nc.scalar.activation(
                            out_t[:, sw * 512 : (sw + 1) * 512], ps[:], Copy,
                            bias=b_sb[:],
                        )
                # V (natural layout, tokens on partitions)
                xtv = []
                for f in range(NF):
                    t = xin.tile([128, NS], BF, tag="xc", name=f"xv{f}")
                    nc.sync.dma_start(t[:], xv_ap[f])
                    xtv.append(t)
                for si in range(NS // 128):
                    psv = psum.tile([128, 128], F32, tag="s", name=f"psv{si}")
                    for f in range(NF):
                        nc.tensor.matmul(
                            psv[:],
                            lhsT=xtv[f][:, si * 128 : (si + 1) * 128],
                            rhs=wv_sb[:, f, :],
                            start=(f == 0),
                            stop=False,
                        )
                    nc.tensor.matmul(
                        psv[:], lhsT=ones1[:], rhs=bv_sb[:], start=False, stop=True
                    )
                    nc.vector.tensor_copy(V0[:, si, 0:HD], psv[:, 0:HD])
                    nc.vector.tensor_copy(V1[:, si, 0:HD], psv[:, HD:128])

            # --- attention + output projection ---
            with (
                tc.tile_pool(name="epool", bufs=6) as epool,
                tc.tile_pool(name="npool", bufs=2) as npool,
                tc.tile_pool(name="ypool", bufs=3) as ypool,
                tc.tile_pool(name="opsum", bufs=2, space="PSUM") as opsum,
            ):
                for b in range(B):
                    for qw in range(2):
                        q0 = b * S + qw * 1024
                        po0 = opsum.tile([65, 1024], F32, tag="o", name=f"po0_{b}{qw}")
                        po1 = opsum.tile([65, 1024], F32, tag="o", name=f"po1_{b}{qw}")
                        for k2t in range(S // 128):
                            si = b * 16 + k2t
                            ks = si * 128
                            ps0 = psum.tile([128, 1024], F32, tag="s",
                                            name=f"ps0_{b}{qw}{k2t}")
                            ps1 = psum.tile([128, 1024], F32, tag="s",
                                            name=f"ps1_{b}{qw}{k2t}")
                            for hf in range(2):
                                qs = q0 + hf * 512
                                fs = slice(hf * 512, (hf + 1) * 512)
                                nc.tensor.matmul(
                                    ps0[:, fs],
                                    lhsT=KT[0:64, ks : ks + 128],
                                    rhs=QT[0:64, qs : qs + 512],
                                    tile_position=(0, 0),
                                )
                                nc.tensor.matmul(
                                    ps1[:, fs],
                                    lhsT=KT[64:128, ks : ks + 128],
                                    rhs=QT[64:128, qs : qs + 512],
                                    tile_position=(64, 0),
                                )
                            e0 = epool.tile([128, 1024], BF, tag="e",
                                            name=f"e0_{b}{qw}{k2t}")
                            e1 = epool.tile([128, 1024], BF, tag="e",
                                            name=f"e1_{b}{qw}{k2t}")
                            nc.scalar.activation(e0[:], ps0[:], Exp, scale=0.125)
                            nc.scalar.activation(e1[:], ps1[:], Exp, scale=0.125)
                            for hf in range(2):
                                fs = slice(hf * 512, (hf + 1) * 512)
                                nc.tensor.matmul(
                                    po0[:, fs], lhsT=V0[:, si, :], rhs=e0[:, fs],
                                    start=(k2t == 0), stop=(k2t == 15),
                                )
                                nc.tensor.matmul(
                                    po1[:, fs], lhsT=V1[:, si, :], rhs=e1[:, fs],
                                    start=(k2t == 0), stop=(k2t == 15),
                                )
                        # normalize: On[hd, q] = O^T[hd, q] / sums[q]
                        # lane-aligned reciprocal (row 64 -> row 64); gpsimd
                        # broadcast handles the partition shift afterwards
                        r0 = npool.tile([65, 1024], F32, tag="r0", name=f"r0_{b}{qw}")
                        r1 = npool.tile([65, 1024], F32, tag="r1", name=f"r1_{b}{qw}")
                        nc.vector.reciprocal(r0[64:65, :], po0[64:65, :])
                        nc.vector.reciprocal(r1[64:65, :], po1[64:65, :])
                        # partition_broadcast only reads base-partition-0 APs;
                        # DMA shifts the row down first
                        rs0 = npool.tile([1, 1024], F32, tag="rs0", name=f"rs0_{b}{qw}")
                        rs1 = npool.tile([1, 1024], F32, tag="rs1", name=f"rs1_{b}{qw}")
                        nc.scalar.dma_start(rs0[:], r0[64:65, :])
                        nc.scalar.dma_start(rs1[:], r1[64:65, :])
                        rb0 = npool.tile([64, 1024], F32, tag="rb0", name=f"rb0_{b}{qw}")
                        rb1 = npool.tile([64, 1024], F32, tag="rb1", name=f"rb1_{b}{qw}")
                        nc.gpsimd.partition_broadcast(rb0[:], rs0[:])
                        nc.gpsimd.partition_broadcast(rb1[:], rs1[:])
                        on = npool.tile([128, 1024], BF, tag="on", name=f"on_{b}{qw}")
                        on1 = npool.tile([64, 1024], BF, tag="on1", name=f"on1_{b}{qw}")
                        nc.vector.tensor_mul(on[0:64, :], po0[0:64, :], rb0[:])
                        nc.vector.tensor_mul(on1[:], po1[0:64, :], rb1[:])
                        nc.scalar.dma_start(on[64:128, :], on1[:])
                        for oc in range(NF):
                            for hf in range(2):
                                fs = slice(hf * 512, (hf + 1) * 512)
                                py = psum.tile([128, 512], F32, tag="s",
                                               name=f"py_{b}{qw}{oc}{hf}")
                                nc.tensor.matmul(
                                    py[:], lhsT=wo_sb[:, oc, :], rhs=on[:, fs]
                                )
                                ysb = ypool.tile([128, 512], F32, tag="y",
                                                 name=f"y_{b}{qw}{oc}{hf}")
                                nc.vector.tensor_copy(ysb[:], py[:])
                                nc.sync.dma_start(
                                    y_ap[oc, :, q0 + hf * 512 : q0 + (hf + 1) * 512],
                                    ysb[:],
                                )
    nc.compile()
    return nc


def _get_nc():
    global _nc
    with _cache:
        if _nc is None:
            _nc = _build_nc()
        return _nc


def kernel(q, k, v, wq_w, wq_b, wk_w, wk_b, wv_w, wv_b, wo_w, wo_b):
    global LAST_RESULT
    nc = _get_nc()

    def xT(a):
        return np.ascontiguousarray(np.asarray(a).reshape(NS, H).astype(BF16).T)

    xq_t, xk_t, xv_t = xT(q), xT(k), xT(v)
    wq_w = np.asarray(wq_w, dtype=np.float32)
    wk_w = np.asarray(wk_w, dtype=np.float32)
    wv_w = np.asarray(wv_w, dtype=np.float32)
    wo_w = np.asarray(wo_w, dtype=np.float32)

    in_maps = []
    for c in range(N_CORES):
        cs = slice(c * CPC, (c + 1) * CPC)
        in_maps.append({
            "xq_t": xq_t,
            "xk_t": xk_t,
            "xv_t": xv_t,
            "wq_t": np.ascontiguousarray(wq_w[cs, :].astype(BF16).T),
            "wk_t": np.ascontiguousarray(wk_w[cs, :].astype(BF16).T),
            "wv_t": np.ascontiguousarray(wv_w[cs, :].astype(BF16).T),
            "bq": np.asarray(wq_b, np.float32)[cs].reshape(CPC, 1),
            "bk": np.asarray(wk_b, np.float32)[cs].reshape(CPC, 1),
            "bv": np.asarray(wv_b, np.float32)[cs].astype(BF16).reshape(1, CPC),
            "wo_t": np.ascontiguousarray(wo_w[:, cs].astype(BF16).T),
        })

    res = run_bass_kernel_spmd(
        nc, in_maps, core_ids=list(range(N_CORES)),
        trace=bool(int(os.environ.get("MHA_TRACE", "0"))),
    )
    LAST_RESULT = res

    y = res.results[0]["y_t"].astype(np.float64)
    for c in range(1, N_CORES):
        y += res.results[c]["y_t"]
    y = y.T + np.asarray(wo_b, np.float64)[None, :]
    return y.reshape(B, S, H).astype(np.float32)



# revision 6
# speedup vs baseline: 1.6537x; 1.6537x over previous
"""Multi-head attention (B=2, S=2048, H=1024, 16 heads) on 8 trn2 NeuronCores.

Sharding: batch(2) x head-group(4) tensor parallel. Core (b, g) owns batch b
and heads 4g..4g+3 (channels 256g..256g+256 of the QKV projections / input
channels of the output projection). Partial wo outputs are summed on host.

Device-side dataflow per core (matmuls bf16, f32 PSUM accumulation):
  QT/KT[c, s]: transposed projections (channels on partitions), bias via K=1
  ones-matmul; V[s, c] natural layout with a ones column per head (row sums).
  Per head-pair p, query-block qb (512 q), key-tile kk (128 k):
    sc[k, 0:512]=h_even scores, sc[k, 512:1024]=h_odd  (row-packed concurrent)
    e = exp(sc/8)  (single [128,1024] ACT instr, both heads)
    po[0:65, 0:512] += V_even_aug . e_even ; po[:, 512:1024] += V_odd_aug ...
  Epilogue: early-drain po->SBUF (frees psum), reciprocal_approx_fast on the
  sums row, DMA-shift to partition 0, gpsimd partition_broadcast, DVE muls.
  wo flipped: y[q, oc] = on_pair0.T @ wo0 + on_pair1.T @ wo1  (queries on
  partitions -> natural-layout bf16 output rows).
"""

import os
import threading

import numpy as np
import ml_dtypes

import concourse.bass as bass
import concourse.mybir as mybir
import concourse.tile as tile
from concourse import bacc
from concourse.bass_utils import run_bass_kernel_spmd

BF16 = ml_dtypes.bfloat16
F32 = mybir.dt.float32
BF = mybir.dt.bfloat16

B = 2
S = 2048
H = 1024
NH = 16
HD = 64
NG = 4              # head groups (TP degree)
HPG = 4             # heads per group
CPG = HPG * HD      # 256 channels per group
NF = H // 128       # 8 input-feature chunks
N_CORES = 8
NKT = S // 128      # 16 key tiles
NQB = S // 512      # 4 query blocks
QB = 512

_cache = threading.Lock()
_nc = None

LAST_RESULT = None  # BassKernelResults of the most recent run (for test.py)


def _build_nc():
    nc = bacc.Bacc(None, target_bir_lowering=False, debug=False)

    xq_d = nc.dram_tensor("xq_t", [H, S], BF, kind="ExternalInput")
    xk_d = nc.dram_tensor("xk_t", [H, S], BF, kind="ExternalInput")
    xv_d = nc.dram_tensor("xv_t", [H, S], BF, kind="ExternalInput")
    wq_d = nc.dram_tensor("wq_t", [H, CPG], BF, kind="ExternalInput")
    wk_d = nc.dram_tensor("wk_t", [H, CPG], BF, kind="ExternalInput")
    wv_d = nc.dram_tensor("wv_t", [H, CPG], BF, kind="ExternalInput")
    bq_d = nc.dram_tensor("bq", [1, CPG], BF, kind="ExternalInput")
    bk_d = nc.dram_tensor("bk", [1, CPG], BF, kind="ExternalInput")
    bv_d = nc.dram_tensor("bv", [1, CPG], BF, kind="ExternalInput")
    wo_d = nc.dram_tensor("wo_t", [CPG, H], BF, kind="ExternalInput")
    y_d = nc.dram_tensor("y_t", [S, H], BF, kind="ExternalOutput")

    xq_ap = xq_d.rearrange("(nf p) s -> nf p s", p=128)
    xk_ap = xk_d.rearrange("(nf p) s -> nf p s", p=128)
    xv_ap = xv_d.rearrange("(nf p) s -> nf p s", p=128)
    y_ap = y_d.rearrange("(nt p) o -> nt p o", p=128)

    Exp = mybir.ActivationFunctionType.Exp

    with tile.TileContext(nc) as tc:
        with (
            tc.tile_pool(name="const", bufs=1) as const,
            tc.tile_pool(name="xpool", bufs=8) as xpool,
            tc.tile_pool(name="res", bufs=1) as res,
            tc.tile_pool(name="epool", bufs=2) as epool,
            tc.tile_pool(name="npool", bufs=2) as npool,
            tc.tile_pool(name="ypool", bufs=3) as ypool,
            tc.tile_pool(name="psum", bufs=1, space="PSUM") as psum,
        ):
            # --- constants / weights ---
            wq_sb = const.tile([128, NF, CPG], BF)
            wk_sb = const.tile([128, NF, CPG], BF)
            wv_sb = const.tile([128, NF, CPG], BF)
            wo_sb = const.tile([128, 2, H], BF)
            bq_sb = const.tile([1, CPG], BF)
            bk_sb = const.tile([1, CPG], BF)
            bv_sb = const.tile([1, CPG], BF)
            ones_t = const.tile([1, QB], BF)
            nc.sync.dma_start(wq_sb[:], wq_d.rearrange("(nf p) c -> p nf c", p=128))
            nc.sync.dma_start(wk_sb[:], wk_d.rearrange("(nf p) c -> p nf c", p=128))
            nc.sync.dma_start(wv_sb[:], wv_d.rearrange("(nf p) c -> p nf c", p=128))
            nc.sync.dma_start(wo_sb[:], wo_d.rearrange("(t p) o -> p t o", p=128))
            nc.sync.dma_start(bq_sb[:], bq_d[:])
            nc.sync.dma_start(bk_sb[:], bk_d[:])
            nc.sync.dma_start(bv_sb[:], bv_d[:])
            nc.gpsimd.memset(ones_t[:], 1.0)

            # --- residents ---
            QT = [res.tile([128, S], BF, name=f"QT{p}") for p in range(2)]
            KT = [res.tile([128, S], BF, name=f"KT{p}") for p in range(2)]
            VA = [res.tile([128, NKT, HD + 1], BF, name=f"VA{h}") for h in range(4)]
            for h in range(4):
                nc.gpsimd.memset(VA[h][:, :, HD : HD + 1], 1.0)

            # --- input staging (issue all loads up front; DMA engine pipelines) ---
            xk_t = []
            xv_t = []
            xq_t = []
            for f in range(NF):
                t = xpool.tile([128, S], BF, tag="xk", name=f"xk{f}")
                nc.sync.dma_start(t[:], xk_ap[f])
                xk_t.append(t)
            for f in range(NF):
                t = xpool.tile([128, S], BF, tag="xv", name=f"xv{f}")
                nc.sync.dma_start(t[:], xv_ap[f])
                xv_t.append(t)
            for f in range(NF):
                t = xpool.tile([128, S], BF, tag="xq", name=f"xq{f}")
                nc.sync.dma_start(t[:], xq_ap[f])
                xq_t.append(t)

            def qk_proj(p, xt, w_sb, b_sb, out_t):
                cs = slice(p * 128, (p + 1) * 128)
                for win in range(S // QB):
                    ws = slice(win * QB, (win + 1) * QB)
                    ps = psum.tile([128, QB], F32, tag="py",
                                   name=f"ps{p}{win}", bufs=2)
                    for f in range(NF):
                        nc.tensor.matmul(
                            ps[:], lhsT=w_sb[:, f, cs], rhs=xt[f][:, ws],
                            start=(f == 0), stop=False,
                        )
                    nc.tensor.matmul(
                        ps[:], lhsT=b_sb[0:1, cs], rhs=ones_t[:],
                        start=False, stop=True,
                    )
                    nc.vector.tensor_copy(out_t[:, ws], ps[:])

            def v_proj(p):
                cs = slice(p * 128, (p + 1) * 128)
                for st in range(NKT):
                    ts = slice(st * 128, (st + 1) * 128)
                    psv = psum.tile([128, 128], F32, tag="py",
                                    name=f"psv{p}{st}", bufs=2)
                    for f in range(NF):
                        nc.tensor.matmul(
                            psv[:], lhsT=xv_t[f][:, ts], rhs=wv_sb[:, f, cs],
                            start=(f == 0), stop=False,
                        )
                    nc.tensor.matmul(
                        psv[:], lhsT=ones_t[0:1, 0:128], rhs=bv_sb[0:1, cs],
                        start=False, stop=True,
                    )
                    nc.vector.tensor_copy(VA[2 * p][:, st, 0:HD], psv[:, 0:HD])
                    nc.vector.tensor_copy(VA[2 * p + 1][:, st, 0:HD], psv[:, HD:128])

            on_tiles = {}

            def attention(p, qb):
                qs = slice(qb * QB, (qb + 1) * QB)
                po = psum.tile([128, 1024], F32, tag="po",
                               name=f"po{p}{qb}", bufs=1)
                for kk in range(NKT):
                    ks = slice(kk * 128, (kk + 1) * 128)
                    sc = psum.tile([128, 1024], F32, tag="sc",
                                   name=f"sc{p}{qb}{kk}", bufs=2)
                    nc.tensor.matmul(
                        sc[:, 0:QB],
                        lhsT=KT[p][0:HD, ks], rhs=QT[p][0:HD, qs],
                        tile_position=(0, 0),
                    )
                    nc.tensor.matmul(
                        sc[:, QB:1024],
                        lhsT=KT[p][HD:128, ks], rhs=QT[p][HD:128, qs],
                        tile_position=(64, 0),
                    )
                    e = epool.tile([128, 1024], BF, tag="e", name=f"e{p}{qb}{kk}")
                    nc.scalar.activation(e[:], sc[:], Exp, scale=0.125)
                    nc.tensor.matmul(
                        po[0:HD + 1, 0:QB], lhsT=VA[2 * p][:, kk, :],
                        rhs=e[:, 0:QB],
                        start=(kk == 0), stop=(kk == NKT - 1),
                    )
                    nc.tensor.matmul(
                        po[0:HD + 1, QB:1024], lhsT=VA[2 * p + 1][:, kk, :],
                        rhs=e[:, QB:1024],
                        start=(kk == 0), stop=(kk == NKT - 1),
                    )
                # epilogue: early-drain po, then normalize
                pou = npool.tile([HD + 1, 1024], F32, tag="pou", name=f"pou{p}{qb}")
                nc.vector.tensor_copy(pou[:], po[0:HD + 1, :])
                rr = npool.tile([HD + 1, 1024], F32, tag="rr", name=f"rr{p}{qb}")
                nc.vector.reciprocal(rr[HD:HD + 1, :], pou[HD:HD + 1, :])
                rs = npool.tile([1, 1024], F32, tag="rs", name=f"rs{p}{qb}")
                nc.sync.dma_start(rs[:], rr[HD:HD + 1, :])
                rb = npool.tile([HD, 1024], F32, tag="rb", name=f"rb{p}{qb}")
                nc.gpsimd.partition_broadcast(rb[:], rs[:])
                on = npool.tile([128, QB], BF, tag="on", name=f"on{p}{qb}", bufs=8)
                ot = npool.tile([HD, QB], BF, tag="ot", name=f"ot{p}{qb}")
                nc.vector.tensor_mul(on[0:HD, :], pou[0:HD, 0:QB], rb[:, 0:QB])
                nc.vector.tensor_mul(ot[:], pou[0:HD, QB:1024], rb[:, QB:1024])
                nc.sync.dma_start(on[HD:128, :], ot[:])
                on_tiles[(p, qb)] = on

            def wo_block(qb):
                on0 = on_tiles[(0, qb)]
                on1 = on_tiles[(1, qb)]
                for qt in range(QB // 128):
                    tqs = slice(qt * 128, (qt + 1) * 128)
                    ti = qb * (QB // 128) + qt
                    ysb = ypool.tile([128, H], BF, tag="y", name=f"y{qb}{qt}")
                    for oc in range(2):
                        ocs = slice(oc * 512, (oc + 1) * 512)
                        py = psum.tile([128, 512], F32, tag="py",
                                       name=f"py{qb}{qt}{oc}", bufs=2)
                        nc.tensor.matmul(
                            py[:], lhsT=on0[:, tqs], rhs=wo_sb[:, 0, ocs],
                            start=True, stop=False,
                        )
                        nc.tensor.matmul(
                            py[:], lhsT=on1[:, tqs], rhs=wo_sb[:, 1, ocs],
                            start=False, stop=True,
                        )
                        nc.vector.tensor_copy(ysb[:, ocs], py[:])
                    nc.sync.dma_start(y_ap[ti], ysb[:])

            # --- emission order == scheduling priority ---
            qk_proj(0, xk_t, wk_sb, bk_sb, KT[0])
            v_proj(0)
            qk_proj(0, xq_t, wq_sb, bq_sb, QT[0])
            for qb in range(NQB):
                attention(0, qb)
            qk_proj(1, xk_t, wk_sb, bk_sb, KT[1])
            v_proj(1)
            qk_proj(1, xq_t, wq_sb, bq_sb, QT[1])
            for qb in range(NQB):
                attention(1, qb)
                wo_block(qb)
    nc.compile()
    return nc


def _get_nc():
    global _nc
    with _cache:
        if _nc is None:
            _nc = _build_nc()
        return _nc


def kernel(q, k, v, wq_w, wq_b, wk_w, wk_b, wv_w, wv_b, wo_w, wo_b):
    global LAST_RESULT
    nc = _get_nc()

    def xT(a, b):
        return np.ascontiguousarray(np.asarray(a)[b].astype(BF16).T)

    wq_w = np.asarray(wq_w, dtype=np.float32)
    wk_w = np.asarray(wk_w, dtype=np.float32)
    wv_w = np.asarray(wv_w, dtype=np.float32)
    wo_w = np.asarray(wo_w, dtype=np.float32)

    xs = {}
    for b in range(B):
        xs[b] = (xT(q, b), xT(k, b), xT(v, b))

    in_maps = []
    for c in range(N_CORES):
        b, g = c // NG, c % NG
        cs = slice(g * CPG, (g + 1) * CPG)
        xq_t, xk_t, xv_t = xs[b]
        in_maps.append({
            "xq_t": xq_t,
            "xk_t": xk_t,
            "xv_t": xv_t,
            "wq_t": np.ascontiguousarray(wq_w[cs, :].astype(BF16).T),
            "wk_t": np.ascontiguousarray(wk_w[cs, :].astype(BF16).T),
            "wv_t": np.ascontiguousarray(wv_w[cs, :].astype(BF16).T),
            "bq": np.asarray(wq_b, np.float32)[cs].astype(BF16).reshape(1, CPG),
            "bk": np.asarray(wk_b, np.float32)[cs].astype(BF16).reshape(1, CPG),
            "bv": np.asarray(wv_b, np.float32)[cs].astype(BF16).reshape(1, CPG),
            "wo_t": np.ascontiguousarray(wo_w[:, cs].astype(BF16).T),
        })

    res = run_bass_kernel_spmd(
        nc, in_maps, core_ids=list(range(N_CORES)),
        trace=bool(int(os.environ.get("MHA_TRACE", "0"))),
    )
    LAST_RESULT = res

    out = np.empty((B, S, H), dtype=np.float32)
    wo_bias = np.asarray(wo_b, np.float32)[None, :]
    for b in range(B):
        acc = res.results[b * NG]["y_t"].astype(np.float32)
        for g in range(1, NG):
            acc += res.results[b * NG + g]["y_t"].astype(np.float32)
        out[b] = acc + wo_bias
    return out


# revision 9
# speedup vs baseline: 1.7395x; 1.0519x over previous
"""Multi-head attention (B=2, S=2048, H=1024, 16 heads) on 8 trn2 NeuronCores.

Sharding: batch(2) x head-group(4) tensor parallel. Core (b, g) owns batch b
and heads 4g..4g+3 (channels 256g..256g+256 of the QKV projections / input
channels of the output projection). Partial wo outputs are summed on host.

Device-side dataflow per core (matmuls bf16, f32 PSUM accumulation):
  QT/KT[c, s]: transposed projections (channels on partitions), bias via K=1
  ones-matmul; V[s, c] natural layout with a ones column per head (row sums).
  Per head-pair p, query-block qb (512 q), key-tile kk (128 k):
    sc[k, 0:512]=h_even scores, sc[k, 512:1024]=h_odd  (row-packed concurrent)
    e = exp(sc/8)  (single [128,1024] ACT instr, both heads)
    po[0:65, 0:512] += V_even_aug . e_even ; po[:, 512:1024] += V_odd_aug ...
  Epilogue: early-drain po->SBUF (frees psum), reciprocal_approx_fast on the
  sums row, DMA-shift to partition 0, gpsimd partition_broadcast, DVE muls.
  wo flipped: y[q, oc] = on_pair0.T @ wo0 + on_pair1.T @ wo1  (queries on
  partitions -> natural-layout bf16 output rows).
"""

import os
import threading

import numpy as np
import ml_dtypes

import concourse.bass as bass
import concourse.mybir as mybir
import concourse.tile as tile
from concourse import bacc
from concourse.bass_utils import run_bass_kernel_spmd

BF16 = ml_dtypes.bfloat16
F32 = mybir.dt.float32
BF = mybir.dt.bfloat16

B = 2
S = 2048
H = 1024
NH = 16
HD = 64
NG = 4              # head groups (TP degree)
HPG = 4             # heads per group
CPG = HPG * HD      # 256 channels per group
NF = H // 128       # 8 input-feature chunks
N_CORES = 8
NKT = S // 128      # 16 key tiles
NQB = S // 512      # 4 query blocks
QB = 512

_cache = threading.Lock()
_nc = None

LAST_RESULT = None  # BassKernelResults of the most recent run (for test.py)


def _build_nc():
    nc = bacc.Bacc(None, target_bir_lowering=False, debug=False)

    xq_d = nc.dram_tensor("xq_t", [H, S], BF, kind="ExternalInput")
    xk_d = nc.dram_tensor("xk_t", [H, S], BF, kind="ExternalInput")
    xv_d = nc.dram_tensor("xv_t", [H, S], BF, kind="ExternalInput")
    wq_d = nc.dram_tensor("wq_t", [H, CPG], BF, kind="ExternalInput")
    wk_d = nc.dram_tensor("wk_t", [H, CPG], BF, kind="ExternalInput")
    wv_d = nc.dram_tensor("wv_t", [H, CPG], BF, kind="ExternalInput")
    bq_d = nc.dram_tensor("bq", [1, CPG], BF, kind="ExternalInput")
    bk_d = nc.dram_tensor("bk", [1, CPG], BF, kind="ExternalInput")
    bv_d = nc.dram_tensor("bv", [1, CPG], BF, kind="ExternalInput")
    wo_d = nc.dram_tensor("wo_t", [CPG, H], BF, kind="ExternalInput")
    y_d = nc.dram_tensor("y_t", [S, H], BF, kind="ExternalOutput")

    xq_ap = xq_d.rearrange("(nf p) s -> nf p s", p=128)
    xk_ap = xk_d.rearrange("(nf p) s -> nf p s", p=128)
    xv_ap = xv_d.rearrange("(nf p) s -> nf p s", p=128)
    y_ap = y_d.rearrange("(nt p) o -> nt p o", p=128)

    Exp = mybir.ActivationFunctionType.Exp

    with tile.TileContext(nc) as tc:
        with (
            tc.tile_pool(name="const", bufs=1) as const,
            tc.tile_pool(name="xpool", bufs=8) as xpool,
            tc.tile_pool(name="res", bufs=1) as res,
            tc.tile_pool(name="epool", bufs=2) as epool,
            tc.tile_pool(name="npool", bufs=2) as npool,
            tc.tile_pool(name="ypool", bufs=3) as ypool,
            tc.tile_pool(name="psum", bufs=1, space="PSUM") as psum,
        ):
            # --- constants / weights ---
            wq_sb = const.tile([128, NF, CPG], BF)
            wk_sb = const.tile([128, NF, CPG], BF)
            wv_sb = const.tile([128, NF, CPG], BF)
            wo_sb = const.tile([128, 2, H], BF)
            bq_sb = const.tile([1, CPG], BF)
            bk_sb = const.tile([1, CPG], BF)
            bv_sb = const.tile([1, CPG], BF)
            ones_t = const.tile([1, QB], BF)
            nc.sync.dma_start(wq_sb[:], wq_d.rearrange("(nf p) c -> p nf c", p=128))
            nc.sync.dma_start(wk_sb[:], wk_d.rearrange("(nf p) c -> p nf c", p=128))
            nc.sync.dma_start(wv_sb[:], wv_d.rearrange("(nf p) c -> p nf c", p=128))
            nc.sync.dma_start(wo_sb[:], wo_d.rearrange("(t p) o -> p t o", p=128))
            nc.sync.dma_start(bq_sb[:], bq_d[:])
            nc.sync.dma_start(bk_sb[:], bk_d[:])
            nc.sync.dma_start(bv_sb[:], bv_d[:])
            nc.gpsimd.memset(ones_t[:], 1.0)

            # --- residents ---
            QT = [res.tile([128, S], BF, name=f"QT{p}") for p in range(2)]
            KT = [res.tile([128, S], BF, name=f"KT{p}") for p in range(2)]
            VA = [res.tile([128, NKT, HD + 1], BF, name=f"VA{h}") for h in range(4)]
            for h in range(4):
                nc.gpsimd.memset(VA[h][:, :, HD : HD + 1], 1.0)

            # --- input staging (issue all loads up front; DMA engine pipelines) ---
            xk_t = []
            xv_t = []
            xq_t = []
            for f in range(NF):
                t = xpool.tile([128, S], BF, tag="xk", name=f"xk{f}")
                nc.sync.dma_start(t[:], xk_ap[f])
                xk_t.append(t)
            for f in range(NF):
                t = xpool.tile([128, S], BF, tag="xv", name=f"xv{f}")
                nc.sync.dma_start(t[:], xv_ap[f])
                xv_t.append(t)
            for f in range(NF):
                t = xpool.tile([128, S], BF, tag="xq", name=f"xq{f}")
                nc.sync.dma_start(t[:], xq_ap[f])
                xq_t.append(t)

            def qk_proj(p, xt, w_sb, b_sb, out_t, wins=None):
                cs = slice(p * 128, (p + 1) * 128)
                for win in (range(S // QB) if wins is None else wins):
                    ws = slice(win * QB, (win + 1) * QB)
                    ps = psum.tile([128, QB], F32, tag="py",
                                   name=f"ps{p}{win}", bufs=2)
                    for f in range(NF):
                        nc.tensor.matmul(
                            ps[:], lhsT=w_sb[:, f, cs], rhs=xt[f][:, ws],
                            start=(f == 0), stop=False,
                        )
                    nc.tensor.matmul(
                        ps[:], lhsT=b_sb[0:1, cs], rhs=ones_t[:],
                        start=False, stop=True,
                    )
                    nc.vector.tensor_copy(out_t[:, ws], ps[:])

            def v_proj_all():
                for st in range(NKT):
                    ts = slice(st * 128, (st + 1) * 128)
                    psv = psum.tile([128, CPG], F32, tag="py",
                                    name=f"psv{st}", bufs=2)
                    for f in range(NF):
                        nc.tensor.matmul(
                            psv[:], lhsT=xv_t[f][:, ts], rhs=wv_sb[:, f, :],
                            start=(f == 0), stop=False,
                        )
                    nc.tensor.matmul(
                        psv[:], lhsT=ones_t[0:1, 0:128], rhs=bv_sb[:],
                        start=False, stop=True,
                    )
                    for h in range(4):
                        nc.vector.tensor_copy(
                            VA[h][:, st, 0:HD], psv[:, h * HD:(h + 1) * HD]
                        )

            on_tiles = {}

            def attention(p, qb):
                qs = slice(qb * QB, (qb + 1) * QB)
                po = psum.tile([128, 1024], F32, tag="po",
                               name=f"po{p}{qb}", bufs=1)
                for kk in range(NKT):
                    ks = slice(kk * 128, (kk + 1) * 128)
                    sc = psum.tile([128, 1024], F32, tag="sc",
                                   name=f"sc{p}{qb}{kk}", bufs=2)
                    nc.tensor.matmul(
                        sc[:, 0:QB],
                        lhsT=KT[p][0:HD, ks], rhs=QT[p][0:HD, qs],
                        tile_position=(0, 0),
                    )
                    nc.tensor.matmul(
                        sc[:, QB:1024],
                        lhsT=KT[p][HD:128, ks], rhs=QT[p][HD:128, qs],
                        tile_position=(64, 0),
                    )
                    e = epool.tile([128, 1024], BF, tag="e", name=f"e{p}{qb}{kk}")
                    nc.scalar.activation(e[:], sc[:], Exp, scale=0.125)
                    nc.tensor.matmul(
                        po[0:HD + 1, 0:QB], lhsT=VA[2 * p][:, kk, :],
                        rhs=e[:, 0:QB],
                        start=(kk == 0), stop=(kk == NKT - 1),
                    )
                    nc.tensor.matmul(
                        po[0:HD + 1, QB:1024], lhsT=VA[2 * p + 1][:, kk, :],
                        rhs=e[:, QB:1024],
                        start=(kk == 0), stop=(kk == NKT - 1),
                    )
                # epilogue: early-drain po, then normalize.
                # reciprocal of the [1,1024] sums row is done spread across 64
                # partitions (DMA repack) -- a [1,N] DVE reciprocal is ~6.5us.
                pou = npool.tile([HD + 1, 1024], F32, tag="pou", name=f"pou{p}{qb}")
                nc.vector.tensor_copy(pou[:], po[0:HD + 1, :])
                rp = npool.tile([64, 16], F32, tag="rp", name=f"rp{p}{qb}")
                nc.sync.dma_start(
                    rp[:], pou[HD:HD + 1, :].rearrange("a (b c) -> a b c", b=64)
                )
                rr = npool.tile([64, 16], F32, tag="rr", name=f"rr{p}{qb}")
                nc.vector.reciprocal(rr[:], rp[:])
                rs = npool.tile([1, 1024], F32, tag="rs", name=f"rs{p}{qb}")
                nc.sync.dma_start(
                    rs[0:1, :].rearrange("a (b c) -> a b c", b=64), rr[:]
                )
                rb = npool.tile([HD, 1024], F32, tag="rb", name=f"rb{p}{qb}")
                nc.gpsimd.partition_broadcast(rb[:], rs[:])
                on = npool.tile([128, QB], BF, tag="on", name=f"on{p}{qb}", bufs=8)
                ot = npool.tile([HD, QB], BF, tag="ot", name=f"ot{p}{qb}")
                nc.vector.tensor_mul(on[0:HD, :], pou[0:HD, 0:QB], rb[:, 0:QB])
                nc.vector.tensor_mul(ot[:], pou[0:HD, QB:1024], rb[:, QB:1024])
                nc.sync.dma_start(on[HD:128, :], ot[:])
                on_tiles[(p, qb)] = on

            def wo_block(qb):
                on0 = on_tiles[(0, qb)]
                on1 = on_tiles[(1, qb)]
                for qt in range(QB // 128):
                    tqs = slice(qt * 128, (qt + 1) * 128)
                    ti = qb * (QB // 128) + qt
                    ysb = ypool.tile([128, H], BF, tag="y", name=f"y{qb}{qt}")
                    for oc in range(2):
                        ocs = slice(oc * 512, (oc + 1) * 512)
                        py = psum.tile([128, 512], F32, tag="py",
                                       name=f"py{qb}{qt}{oc}", bufs=2)
                        nc.tensor.matmul(
                            py[:], lhsT=on0[:, tqs], rhs=wo_sb[:, 0, ocs],
                            start=True, stop=False,
                        )
                        nc.tensor.matmul(
                            py[:], lhsT=on1[:, tqs], rhs=wo_sb[:, 1, ocs],
                            start=False, stop=True,
                        )
                        nc.vector.tensor_copy(ysb[:, ocs], py[:])
                    nc.sync.dma_start(y_ap[ti], ysb[:])

            # --- emission order == scheduling priority ---
            qk_proj(0, xk_t, wk_sb, bk_sb, KT[0])
            v_proj_all()
            for qb in range(NQB):
                qk_proj(0, xq_t, wq_sb, bq_sb, QT[0], wins=[qb])
                attention(0, qb)
            qk_proj(1, xk_t, wk_sb, bk_sb, KT[1])
            for qb in range(NQB):
                qk_proj(1, xq_t, wq_sb, bq_sb, QT[1], wins=[qb])
                attention(1, qb)
                wo_block(qb)
    nc.compile()
    return nc


def _get_nc():
    global _nc
    with _cache:
        if _nc is None:
            _nc = _build_nc()
        return _nc


def kernel(q, k, v, wq_w, wq_b, wk_w, wk_b, wv_w, wv_b, wo_w, wo_b):
    global LAST_RESULT
    nc = _get_nc()

    def xT(a, b):
        return np.ascontiguousarray(np.asarray(a)[b].astype(BF16).T)

    wq_w = np.asarray(wq_w, dtype=np.float32)
    wk_w = np.asarray(wk_w, dtype=np.float32)
    wv_w = np.asarray(wv_w, dtype=np.float32)
    wo_w = np.asarray(wo_w, dtype=np.float32)

    xs = {}
    for b in range(B):
        xs[b] = (xT(q, b), xT(k, b), xT(v, b))

    in_maps = []
    for c in range(N_CORES):
        b, g = c // NG, c % NG
        cs = slice(g * CPG, (g + 1) * CPG)
        xq_t, xk_t, xv_t = xs[b]
        in_maps.append({
            "xq_t": xq_t,
            "xk_t": xk_t,
            "xv_t": xv_t,
            "wq_t": np.ascontiguousarray(wq_w[cs, :].astype(BF16).T),
            "wk_t": np.ascontiguousarray(wk_w[cs, :].astype(BF16).T),
            "wv_t": np.ascontiguousarray(wv_w[cs, :].astype(BF16).T),
            "bq": np.asarray(wq_b, np.float32)[cs].astype(BF16).reshape(1, CPG),
            "bk": np.asarray(wk_b, np.float32)[cs].astype(BF16).reshape(1, CPG),
            "bv": np.asarray(wv_b, np.float32)[cs].astype(BF16).reshape(1, CPG),
            "wo_t": np.ascontiguousarray(wo_w[:, cs].astype(BF16).T),
        })

    res = run_bass_kernel_spmd(
        nc, in_maps, core_ids=list(range(N_CORES)),
        trace=bool(int(os.environ.get("MHA_TRACE", "0"))),
    )
    LAST_RESULT = res

    out = np.empty((B, S, H), dtype=np.float32)
    wo_bias = np.asarray(wo_b, np.float32)[None, :]
    for b in range(B):
        acc = res.results[b * NG]["y_t"].astype(np.float32)
        for g in range(1, NG):
            acc += res.results[b * NG + g]["y_t"].astype(np.float32)
        out[b] = acc + wo_bias
    return out


# revision 13
# speedup vs baseline: 1.9562x; 1.1246x over previous
"""Multi-head attention (B=2, S=2048, H=1024, 16 heads) on 8 trn2 NeuronCores.

Sharding: batch(2) x head-group(4) tensor parallel. Core (b, g) owns batch b
and heads 4g..4g+3 (channels 256g..256g+256 of the QKV projections / input
channels of the output projection). Partial wo outputs are summed on host.

Device-side dataflow per core (matmuls bf16, f32 PSUM accumulation):
  QT/KT[c, s]: transposed projections (channels on partitions), bias via K=1
  ones-matmul; V[s, c] natural layout with a ones column per head (row sums).
  Per head-pair p, query-block qb (512 q), key-tile kk (128 k):
    sc[k, 0:512]=h_even scores, sc[k, 512:1024]=h_odd  (row-packed concurrent)
    e = exp(sc/8)  (single [128,1024] ACT instr, both heads)
    po[0:65, 0:512] += V_even_aug . e_even ; po[:, 512:1024] += V_odd_aug ...
  Epilogue: early-drain po->SBUF (frees psum), reciprocal_approx_fast on the
  sums row, DMA-shift to partition 0, gpsimd partition_broadcast, DVE muls.
  wo flipped: y[q, oc] = on_pair0.T @ wo0 + on_pair1.T @ wo1  (queries on
  partitions -> natural-layout bf16 output rows).
"""

import os
import threading

import numpy as np
import ml_dtypes

import concourse.bass as bass
import concourse.mybir as mybir
import concourse.tile as tile
from concourse import bacc
from concourse.bass_utils import run_bass_kernel_spmd

BF16 = ml_dtypes.bfloat16
F32 = mybir.dt.float32
BF = mybir.dt.bfloat16

B = 2
S = 2048
H = 1024
NH = 16
HD = 64
NG = 4              # head groups (TP degree)
HPG = 4             # heads per group
CPG = HPG * HD      # 256 channels per group
NF = H // 128       # 8 input-feature chunks
N_CORES = 8
NKT = S // 128      # 16 key tiles
NQB = S // 512      # 4 query blocks
QB = 512

_cache = threading.Lock()
_nc = None

LAST_RESULT = None  # BassKernelResults of the most recent run (for test.py)


def _build_nc():
    nc = bacc.Bacc(None, target_bir_lowering=False, debug=False)

    xq_d = nc.dram_tensor("xq_t", [H, S], BF, kind="ExternalInput")
    xk_d = nc.dram_tensor("xk_t", [H, S], BF, kind="ExternalInput")
    xv_d = nc.dram_tensor("xv_t", [H, S], BF, kind="ExternalInput")
    wq_d = nc.dram_tensor("wq_t", [H, CPG], BF, kind="ExternalInput")
    wk_d = nc.dram_tensor("wk_t", [H, CPG], BF, kind="ExternalInput")
    wv_d = nc.dram_tensor("wv_t", [H, CPG], BF, kind="ExternalInput")
    bq_d = nc.dram_tensor("bq", [1, CPG], BF, kind="ExternalInput")
    bk_d = nc.dram_tensor("bk", [1, CPG], BF, kind="ExternalInput")
    bv_d = nc.dram_tensor("bv", [1, CPG], BF, kind="ExternalInput")
    wo_d = nc.dram_tensor("wo_t", [CPG, H], BF, kind="ExternalInput")
    y_d = nc.dram_tensor("y_t", [S, H], BF, kind="ExternalOutput")

    xq_ap = xq_d.rearrange("(nf p) s -> nf p s", p=128)
    xk_ap = xk_d.rearrange("(nf p) s -> nf p s", p=128)
    xv_ap = xv_d.rearrange("(nf p) s -> nf p s", p=128)
    y_ap = y_d.rearrange("(nt p) o -> nt p o", p=128)

    Exp = mybir.ActivationFunctionType.Exp

    with tile.TileContext(nc) as tc:
        with (
            tc.tile_pool(name="const", bufs=1) as const,
            tc.tile_pool(name="xpool", bufs=8) as xpool,
            tc.tile_pool(name="res", bufs=1) as res,
            tc.tile_pool(name="epool", bufs=2) as epool,
            tc.tile_pool(name="npool", bufs=2) as npool,
            tc.tile_pool(name="ypool", bufs=3) as ypool,
            tc.tile_pool(name="psum", bufs=1, space="PSUM") as psum,
        ):
            # --- constants / weights ---
            wq_sb = const.tile([128, NF, CPG], BF)
            wk_sb = const.tile([128, NF, CPG], BF)
            wv_sb = const.tile([128, NF, CPG], BF)
            wo_sb = const.tile([128, 2, H], BF)
            bq_sb = const.tile([1, CPG], BF)
            bk_sb = const.tile([1, CPG], BF)
            bv_sb = const.tile([1, CPG], BF)
            ones_t = const.tile([1, QB], BF)
            nc.sync.dma_start(wq_sb[:], wq_d.rearrange("(nf p) c -> p nf c", p=128))
            nc.sync.dma_start(wk_sb[:], wk_d.rearrange("(nf p) c -> p nf c", p=128))
            nc.sync.dma_start(wv_sb[:], wv_d.rearrange("(nf p) c -> p nf c", p=128))
            nc.sync.dma_start(wo_sb[:], wo_d.rearrange("(t p) o -> p t o", p=128))
            nc.sync.dma_start(bq_sb[:], bq_d[:])
            nc.sync.dma_start(bk_sb[:], bk_d[:])
            nc.sync.dma_start(bv_sb[:], bv_d[:])
            nc.gpsimd.memset(ones_t[:], 1.0)

            # --- residents ---
            QT = [res.tile([128, S], BF, name=f"QT{p}") for p in range(2)]
            KT = [res.tile([128, S], BF, name=f"KT{p}") for p in range(2)]
            VA = [res.tile([128, NKT, HD + 1], BF, name=f"VA{h}") for h in range(4)]
            for h in range(4):
                nc.gpsimd.memset(VA[h][:, :, HD : HD + 1], 1.0)

            # --- input staging (issue all loads up front; DMA engine pipelines) ---
            xk_t = []
            xv_t = []
            xq_t = []
            for f in range(NF):
                t = xpool.tile([128, S], BF, tag="xk", name=f"xk{f}")
                nc.sync.dma_start(t[:], xk_ap[f])
                xk_t.append(t)
            for f in range(NF):
                t = xpool.tile([128, S], BF, tag="xv", name=f"xv{f}")
                nc.sync.dma_start(t[:], xv_ap[f])
                xv_t.append(t)
            for f in range(NF):
                t = xpool.tile([128, S], BF, tag="xq", name=f"xq{f}")
                nc.sync.dma_start(t[:], xq_ap[f])
                xq_t.append(t)

            def qk_proj(p, xt, w_sb, b_sb, out_t, wins=None):
                cs = slice(p * 128, (p + 1) * 128)
                for win in (range(S // QB) if wins is None else wins):
                    ws = slice(win * QB, (win + 1) * QB)
                    ps = psum.tile([128, QB], F32, tag="sc",
                                   name=f"ps{p}{win}", bufs=2)
                    for f in range(NF):
                        nc.tensor.matmul(
                            ps[:], lhsT=w_sb[:, f, cs], rhs=xt[f][:, ws],
                            start=(f == 0), stop=False,
                        )
                    nc.tensor.matmul(
                        ps[:], lhsT=b_sb[0:1, cs], rhs=ones_t[:],
                        start=False, stop=True,
                    )
                    nc.vector.tensor_copy(out_t[:, ws], ps[:])

            def v_proj_st(st):
                ts = slice(st * 128, (st + 1) * 128)
                psv = psum.tile([128, CPG], F32, tag="py",
                                name=f"psv{st}", bufs=2)
                for f in range(NF):
                    nc.tensor.matmul(
                        psv[:], lhsT=xv_t[f][:, ts], rhs=wv_sb[:, f, :],
                        start=(f == 0), stop=False,
                    )
                nc.tensor.matmul(
                    psv[:], lhsT=ones_t[0:1, 0:128], rhs=bv_sb[:],
                    start=False, stop=True,
                )
                for h in range(4):
                    nc.vector.tensor_copy(
                        VA[h][:, st, 0:HD], psv[:, h * HD:(h + 1) * HD]
                    )

            on_tiles = {}

            def attention(p, qb, pre_kk=None):
                qs = slice(qb * QB, (qb + 1) * QB)
                po = psum.tile([128, 1024], F32, tag="po",
                               name=f"po{p}{qb}", bufs=1)
                for kk in range(NKT):
                    if pre_kk is not None:
                        pre_kk(kk)
                    ks = slice(kk * 128, (kk + 1) * 128)
                    sc = psum.tile([128, 1024], F32, tag="sc",
                                   name=f"sc{p}{qb}{kk}", bufs=2)
                    nc.tensor.matmul(
                        sc[:, 0:QB],
                        lhsT=KT[p][0:HD, ks], rhs=QT[p][0:HD, qs],
                        tile_position=(0, 0),
                    )
                    nc.tensor.matmul(
                        sc[:, QB:1024],
                        lhsT=KT[p][HD:128, ks], rhs=QT[p][HD:128, qs],
                        tile_position=(64, 0),
                    )
                    e = epool.tile([128, 1024], BF, tag="e", name=f"e{p}{qb}{kk}")
                    nc.scalar.activation(e[:], sc[:], Exp, scale=0.125)
                    nc.tensor.matmul(
                        po[0:HD + 1, 0:QB], lhsT=VA[2 * p][:, kk, :],
                        rhs=e[:, 0:QB],
                        start=(kk == 0), stop=(kk == NKT - 1),
                    )
                    nc.tensor.matmul(
                        po[0:HD + 1, QB:1024], lhsT=VA[2 * p + 1][:, kk, :],
                        rhs=e[:, QB:1024],
                        start=(kk == 0), stop=(kk == NKT - 1),
                    )
                # epilogue: early-drain po, then normalize.
                # reciprocal of the [1,1024] sums row is done spread across 64
                # partitions (DMA repack) -- a [1,N] DVE reciprocal is ~6.5us.
                pou = npool.tile([HD + 1, 1024], F32, tag="pou", name=f"pou{p}{qb}")
                nc.vector.tensor_copy(pou[:], po[0:HD + 1, :])
                rp = npool.tile([64, 16], F32, tag="rp", name=f"rp{p}{qb}")
                nc.sync.dma_start(
                    rp[:], pou[HD:HD + 1, :].rearrange("a (b c) -> a b c", b=64)
                )
                rr = npool.tile([64, 16], F32, tag="rr", name=f"rr{p}{qb}")
                nc.vector.reciprocal(rr[:], rp[:])
                rs = npool.tile([1, 1024], F32, tag="rs", name=f"rs{p}{qb}")
                nc.sync.dma_start(
                    rs[0:1, :].rearrange("a (b c) -> a b c", b=64), rr[:]
                )
                rb = npool.tile([HD, 1024], F32, tag="rb", name=f"rb{p}{qb}")
                nc.gpsimd.partition_broadcast(rb[:], rs[:])
                on = npool.tile([128, QB], BF, tag="on", name=f"on{p}{qb}", bufs=8)
                ot = npool.tile([HD, QB], BF, tag="ot", name=f"ot{p}{qb}")
                nc.vector.tensor_mul(on[0:HD, :], pou[0:HD, 0:QB], rb[:, 0:QB])
                nc.vector.tensor_mul(ot[:], pou[0:HD, QB:1024], rb[:, QB:1024])
                nc.sync.dma_start(on[HD:128, :], ot[:])
                on_tiles[(p, qb)] = on

            def wo_block(qb):
                on0 = on_tiles[(0, qb)]
                on1 = on_tiles[(1, qb)]
                for qt in range(QB // 128):
                    tqs = slice(qt * 128, (qt + 1) * 128)
                    ti = qb * (QB // 128) + qt
                    ysb = ypool.tile([128, H], BF, tag="y", name=f"y{qb}{qt}")
                    for oc in range(2):
                        ocs = slice(oc * 512, (oc + 1) * 512)
                        py = psum.tile([128, 512], F32, tag="py",
                                       name=f"py{qb}{qt}{oc}", bufs=2)
                        nc.tensor.matmul(
                            py[:], lhsT=on0[:, tqs], rhs=wo_sb[:, 0, ocs],
                            start=True, stop=False,
                        )
                        nc.tensor.matmul(
                            py[:], lhsT=on1[:, tqs], rhs=wo_sb[:, 1, ocs],
                            start=False, stop=True,
                        )
                        nc.vector.tensor_copy(ysb[:, ocs], py[:])
                    nc.sync.dma_start(y_ap[ti], ysb[:])

            # --- emission order == scheduling priority ---
            qk_proj(0, xk_t, wk_sb, bk_sb, KT[0])
            for st in range(4):
                v_proj_st(st)
            qk_proj(0, xq_t, wq_sb, bq_sb, QT[0], wins=[0])

            def vproj_ahead(kk):
                if kk + 4 < NKT:
                    v_proj_st(kk + 4)

            for qb in range(NQB):
                if qb > 0:
                    qk_proj(0, xq_t, wq_sb, bq_sb, QT[0], wins=[qb])
                attention(0, qb, pre_kk=vproj_ahead if qb == 0 else None)
            qk_proj(1, xk_t, wk_sb, bk_sb, KT[1])
            for qb in range(NQB):
                qk_proj(1, xq_t, wq_sb, bq_sb, QT[1], wins=[qb])
                attention(1, qb)
                wo_block(qb)
    nc.compile()
    return nc


def _get_nc():
    global _nc
    with _cache:
        if _nc is None:
            _nc = _build_nc()
        return _nc


def kernel(q, k, v, wq_w, wq_b, wk_w, wk_b, wv_w, wv_b, wo_w, wo_b):
    global LAST_RESULT
    nc = _get_nc()

    def xT(a, b):
        return np.ascontiguousarray(np.asarray(a)[b].astype(BF16).T)

    wq_w = np.asarray(wq_w, dtype=np.float32)
    wk_w = np.asarray(wk_w, dtype=np.float32)
    wv_w = np.asarray(wv_w, dtype=np.float32)
    wo_w = np.asarray(wo_w, dtype=np.float32)

    xs = {}
    for b in range(B):
        xs[b] = (xT(q, b), xT(k, b), xT(v, b))

    in_maps = []
    for c in range(N_CORES):
        b, g = c // NG, c % NG
        cs = slice(g * CPG, (g + 1) * CPG)
        xq_t, xk_t, xv_t = xs[b]
        in_maps.append({
            "xq_t": xq_t,
            "xk_t": xk_t,
            "xv_t": xv_t,
            "wq_t": np.ascontiguousarray(wq_w[cs, :].astype(BF16).T),
            "wk_t": np.ascontiguousarray(wk_w[cs, :].astype(BF16).T),
            "wv_t": np.ascontiguousarray(wv_w[cs, :].astype(BF16).T),
            "bq": np.asarray(wq_b, np.float32)[cs].astype(BF16).reshape(1, CPG),
            "bk": np.asarray(wk_b, np.float32)[cs].astype(BF16).reshape(1, CPG),
            "bv": np.asarray(wv_b, np.float32)[cs].astype(BF16).reshape(1, CPG),
            "wo_t": np.ascontiguousarray(wo_w[:, cs].astype(BF16).T),
        })

    res = run_bass_kernel_spmd(
        nc, in_maps, core_ids=list(range(N_CORES)),
        trace=bool(int(os.environ.get("MHA_TRACE", "0"))),
    )
    LAST_RESULT = res

    out = np.empty((B, S, H), dtype=np.float32)
    wo_bias = np.asarray(wo_b, np.float32)[None, :]
    for b in range(B):
        acc = res.results[b * NG]["y_t"].astype(np.float32)
        for g in range(1, NG):
            acc += res.results[b * NG + g]["y_t"].astype(np.float32)
        out[b] = acc + wo_bias
    return out
